# revision 38
# baseline (speedup 1.0000x reference)
"""Trainium2 Bass kernel for gnn_message_passing (nn_Conv_82506321756833).

Computes, for N=50000 nodes / E=800000 edges / H=128:
    xp   = gelu(x @ W1 + b1)
    aggr = segment_sum(xp[src] * bases, dst)
    x    = x_feat + aggr
    y    = gelu(bn1(x @ W2 + b2)); y = gelu(bn2(y @ W3 + b3))
    out  = x + y

Sharding: nodes are partitioned contiguously across 8 cores (graph
parallel); each core owns its node shard and all edges whose dst lands
in the shard.  Edges are bucketed host-side by (src range, dst window):
R=4 source ranges x 128-node destination windows.

Phase A (replicated per core): stream x feature-major, compute
xp = gelu(x@W1+b1) for ALL nodes, store as a row table [NA, H] f16 in
device DRAM (a DRAM tile, so the tile framework tracks the per-range
RAW dependencies into the gathers via AP intersection).

Phase B runs R scatter passes; pass r only needs xp rows of range r, so
pass-0 gathers start as soon as the (smallest) first range of phase A
is stored.
Per (range, window) group:
  1. dma_gather xp rows EDGE-major (transpose=False; transpose gathers
     corrupt on swdge queues != 0) on 4 SWDGE queues round-robin.
     Pad slots gather row 0 of the range (bases there are zero); a
     host-side greedy balancer packs every (window, range) cell to just
     under its tile multiple so padding stays ~3%.
  2. multiply by the (host-presorted, edge-major) bases tile -> msg;
  3. scatter-sum via one-hot matmuls msg.T @ S into PSUM; S is built by
     one DVE is_equal per group comparing a broadcast iota row against
     the per-edge dst-slot id;
  4. accumulate PSUM into an SBUF aggregator strip [128, NPAD] f32
     (pass 0 also folds in the residual x).
The final pass fuses the 2-layer FFN (BN folded into W2/W3 + bias) and
streams outputs to HBM in fp16, one store per 8-window group.
"""

import numpy as np

import concourse.bacc as bacc
import concourse.tile as tile
from concourse import mybir

H = 128
WIN = 128
R = 4  # source ranges (rows per range < 32768 for int16 gather indices)
DEBUG_DUMP = False
PRIME_VAL = 0.0
NSWQ = 4  # SWDGE queues for gathers
SCRATCH = 32768  # SWDGE descriptor scratch (bytes/partition)
BN_EPS = 1e-5
F16 = mybir.dt.float16
F32 = mybir.dt.float32
I16 = mybir.dt.int16
U32 = mybir.dt.uint32
GELU = mybir.ActivationFunctionType.Gelu
IS_EQ = mybir.AluOpType.is_equal


def _ceil_to(x, m):
    return (x + m - 1) // m * m


def _wrap16(idx):
    """[L] int16 index list -> [128, L//16] wrapped+replicated."""
    L = idx.shape[0]
    m = idx.reshape(L // 16, 16).T  # [16, L/16]
    return np.ascontiguousarray(np.tile(m, (8, 1)))  # [128, L/16]


TPW = (3, 4, 5, 5)  # target tiles per (window, range) group: ~6% slack
#  per range for the greedy; range 0 smallest so phase A unblocks the
#  first gathers as early as possible


def _range_parts(NB):
    wts = np.array(TPW, np.float64)
    wts /= wts.sum()
    parts = np.floor(NB * wts).astype(np.int64)
    parts[0] += NB - parts.sum()
    starts = np.zeros(R + 1, np.int64)
    np.cumsum(parts, out=starts[1:])
    return parts.tolist(), starts * 128  # tile counts, row starts


def prep_inputs(x_feat, bases, src, dst, W1, b1, W2, b2, W3, b3,
                g1, be1, m1, v1, g2, be2, m2, v2, ncores=8):
    """Host-side sharding: bucket edges by (src range, dst window), build
    per-core tile-grid inputs (bases, slot ids, gather indices, counts)."""
    N = x_feat.shape[0]
    assert N % ncores == 0
    NSH = N // ncores
    NW = (NSH + WIN - 1) // WIN
    NPAD = NW * WIN
    NA = _ceil_to(N, 128)
    NB = NA // 128
    parts, row_start = _range_parts(NB)
    assert max(parts) * 128 <= 32768

    x_feat = np.asarray(x_feat, np.float32)
    bases = np.asarray(bases, np.float32)
    src = np.asarray(src, np.int64)
    dst = np.asarray(dst, np.int64)

    NG = R * NW

    # Greedy range assignment (quota-normalized): place each node's rows in
    # the range where its (core, window) cells stay lowest relative to fair
    # share, flattening the max-over-cores per-cell counts the shared
    # program pads to tile multiples.
    cell = (dst // NSH) * NW + (dst % NSH) // WIN
    order0 = np.argsort(src, kind="stable")
    s_sorted = src[order0]
    c_sorted = cell[order0]
    starts = np.searchsorted(s_sorted, np.arange(N))
    ends = np.searchsorted(s_sorted, np.arange(N) + 1)
    tot = np.bincount(cell, minlength=ncores * NW).astype(np.float64)
    qr = np.array([p / float(NB) for p in parts])  # range share
    quota = np.maximum(tot[None, :] * qr[:, None], 1.0)  # [R, cells]
    cnt_rc = np.zeros((R, ncores * NW))
    used = np.zeros(R, np.int64)
    cap = np.array([p * 128 for p in parts], np.int64)
    assign = np.zeros(N, np.int8)
    for n in np.argsort(-(ends - starts), kind="stable"):
        uc, mult = np.unique(c_sorted[starts[n]:ends[n]], return_counts=True)
        if len(uc):
            d = ((cnt_rc[:, uc] + mult[None, :]) / quota[:, uc]).max(axis=1)
        else:
            d = np.zeros(R)
        d = np.where(used < cap, d, np.inf)
        rpick = int(np.argmin(d))
        assign[n] = rpick
        used[rpick] += 1
        if len(uc):
            cnt_rc[rpick, uc] += mult
    rowof = np.empty(N, np.int64)
    for rr in range(R):
        ids = np.nonzero(assign == rr)[0]
        rowof[ids] = row_start[rr] + np.arange(len(ids))

    xfa = np.zeros((H, NA), np.float16)
    xfa[:, rowof] = x_feat.T.astype(np.float16)

    w1h = np.ascontiguousarray(np.asarray(W1, np.float32).astype(np.float16))
    a1 = (np.asarray(g1, np.float32) /
          np.sqrt(np.asarray(v1, np.float32) + BN_EPS))
    a2 = (np.asarray(g2, np.float32) /
          np.sqrt(np.asarray(v2, np.float32) + BN_EPS))
    w2f = np.ascontiguousarray((np.asarray(W2, np.float32) * a1[None, :])
                               .astype(np.float16))
    w3f = np.ascontiguousarray((np.asarray(W3, np.float32) * a2[None, :])
                               .astype(np.float16))
    c2 = ((np.asarray(b2, np.float32) - np.asarray(m1, np.float32)) * a1
          + np.asarray(be1, np.float32)).astype(np.float32).reshape(H, 1)
    c3 = ((np.asarray(b3, np.float32) - np.asarray(m2, np.float32)) * a2
          + np.asarray(be2, np.float32)).astype(np.float32).reshape(H, 1)
    have_b1 = bool(np.any(np.asarray(b1)))
    b1h = np.asarray(b1, np.float32).astype(np.float16).reshape(1, H)

    # Pass 1: per-core bucketing by (range, window) in range-major order.
    core_of = dst // NSH
    percore = []
    cnt_all = np.zeros((ncores, NG), np.int64)
    for k in range(ncores):
        sel = np.nonzero(core_of == k)[0]
        ld = dst[sel] - k * NSH
        w = ld // WIN
        j = ld % WIN
        s = rowof[src[sel]]
        r = assign[src[sel]].astype(np.int64)
        key = r * NW + w
        order = np.lexsort((s, key))
        w, j, s, r, key, sel = (w[order], j[order], s[order], r[order],
                                key[order], sel[order])
        cntg = np.bincount(key, minlength=NG)
        cnt_all[k] = cntg
        starts2 = np.zeros(NG, np.int64)
        np.cumsum(cntg[:-1], out=starts2[1:])
        rank = np.arange(len(key)) - starts2[key]
        percore.append((key, j, s, r, rank, sel))

    # Shared (max-over-cores) tile grid: all cores run one program.
    T = (cnt_all.max(axis=0) + 127) // 128  # [NG]
    OFFG = np.zeros(NG + 1, np.int64)
    np.cumsum(T, out=OFFG[1:])
    GT = int(OFFG[-1])

    # Pass 2: per-core arrays in the shared grid.
    in_maps = []
    for k in range(ncores):
        key, j, s, r, rank, sel = percore[k]
        gt = OFFG[key] + rank // 128
        p = rank % 128

        basf = np.zeros((128, GT * H), np.float16)
        basf.reshape(128, GT, H)[p, gt, :] = bases[sel].astype(np.float16)
        jd = np.full((128, GT), -1, np.float16)
        jd[p, gt] = j.astype(np.float16)

        ixflat = np.zeros(GT * 128, np.int16)
        ixflat[gt * 128 + p] = (s - row_start[r]).astype(np.int16)
        ixd = np.zeros((128, GT * 8), np.int16)
        for g in range(NG):
            if T[g] == 0:
                continue
            seg = ixflat[OFFG[g] * 128:(OFFG[g] + T[g]) * 128]
            ixd[:, OFFG[g] * 8:(OFFG[g] + T[g]) * 8] = _wrap16(seg)

        xfm = np.zeros((H, NPAD), np.float16)
        xfm[:, :NSH] = x_feat[k * NSH:(k + 1) * NSH].T.astype(np.float16)

        maps = dict(xfa=xfa, basf=basf, jd=jd, ixd=ixd,
                    w1=w1h, w2=w2f, w3=w3f, c2=c2, c3=c3, xfm=xfm)
        if have_b1:
            maps["b1"] = b1h
        in_maps.append(maps)

    meta = dict(N=N, NSH=NSH, NW=NW, NPAD=NPAD, NA=NA, NB=NB,
                T=T.tolist(), OFFG=OFFG.tolist(), GT=GT,
                parts=parts, row_start=row_start.tolist(),
                have_b1=have_b1)
    return in_maps, meta


def build_program(meta, ncores=8, act=GELU):
    NA, NW, NPAD, NB = meta["NA"], meta["NW"], meta["NPAD"], meta["NB"]
    T, OFFG, GT = meta["T"], meta["OFFG"], meta["GT"]
    row_start = meta["row_start"]
    have_b1 = meta["have_b1"]

    nc = bacc.Bacc("TRN2", target_bir_lowering=False, debug=False,
                   num_devices=ncores, num_swdge_queues=NSWQ,
                   dynamic_dma_scratch_size=SCRATCH)
    xfa = nc.dram_tensor("xfa", [H, NA], F16, kind="ExternalInput").ap()
    xfm = nc.dram_tensor("xfm", [H, NPAD], F16, kind="ExternalInput").ap()
    basf = nc.dram_tensor("basf", [128, GT * H], F16,
                          kind="ExternalInput").ap()
    jdd = nc.dram_tensor("jd", [128, GT], F16, kind="ExternalInput").ap()
    ixdd = nc.dram_tensor("ixd", [128, GT * 8], I16,
                          kind="ExternalInput").ap()
    w1 = nc.dram_tensor("w1", [H, H], F16, kind="ExternalInput").ap()
    w2 = nc.dram_tensor("w2", [H, H], F16, kind="ExternalInput").ap()
    w3 = nc.dram_tensor("w3", [H, H], F16, kind="ExternalInput").ap()
    c2 = nc.dram_tensor("c2", [H, 1], F32, kind="ExternalInput").ap()
    c3 = nc.dram_tensor("c3", [H, 1], F32, kind="ExternalInput").ap()
    b1 = (nc.dram_tensor("b1", [1, H], F16, kind="ExternalInput").ap()
          if have_b1 else None)
    outd = nc.dram_tensor("out", [H, NPAD], F16, kind="ExternalOutput").ap()

    dbg = {}
    if DEBUG_DUMP:
        MAXNT0 = max(T) if T else 1
        dbg["g"] = nc.dram_tensor("dbg_g", [128, MAXNT0 * H], F16,
                                  kind="ExternalOutput").ap()
        dbg["msg"] = nc.dram_tensor("dbg_msg", [128, MAXNT0 * H], F16,
                                    kind="ExternalOutput").ap()
        dbg["s"] = nc.dram_tensor("dbg_s", [128, MAXNT0 * 128], F16,
                                  kind="ExternalOutput").ap()
        dbg["ag"] = nc.dram_tensor("dbg_ag", [128, 128], F32,
                                   kind="ExternalOutput").ap()

    swq = [0]

    with tile.TileContext(nc) as tc:
        with (
            tc.tile_pool(name="const", bufs=1) as cpool,
            tc.tile_pool(name="xpd", bufs=1, space="DRAM") as xpdp,
            tc.tile_pool(name="xa", bufs=4) as xap,
            tc.tile_pool(name="xps", bufs=4) as xpsp,
            tc.tile_pool(name="bas", bufs=6) as basp,
            tc.tile_pool(name="gat", bufs=12) as gatp,
            tc.tile_pool(name="msg", bufs=8) as msgp,
            tc.tile_pool(name="st", bufs=8) as stp,
            tc.tile_pool(name="ffn", bufs=2) as ffnp,
            tc.tile_pool(name="og", bufs=2) as ogp,
            tc.tile_pool(name="pxp", bufs=3, space="PSUM") as pxp,
            tc.tile_pool(name="pag", bufs=3, space="PSUM") as pag,
            tc.tile_pool(name="pffn", bufs=2, space="PSUM") as pffn,
        ):
            # constants / resident inputs (big ones on the idle vector queue)
            w1t = cpool.tile([H, H], F16, tag="w1")
            nc.sync.dma_start(w1t[:], w1[:])
            w2t = cpool.tile([H, H], F16, tag="w2")
            nc.sync.dma_start(w2t[:], w2[:])
            w3t = cpool.tile([H, H], F16, tag="w3")
            nc.sync.dma_start(w3t[:], w3[:])
            c2t = cpool.tile([H, 1], F32, tag="c2")
            nc.sync.dma_start(c2t[:], c2[:])
            c3t = cpool.tile([H, 1], F32, tag="c3")
            nc.sync.dma_start(c3t[:], c3[:])
            iota_t = cpool.tile([128, 128], F16, tag="iota")
            nc.gpsimd.iota(iota_t[:], [[1, 128]], channel_multiplier=0,
                           allow_small_or_imprecise_dtypes=True)
            if have_b1:
                b1t = cpool.tile([1, H], F16, tag="b1")
                nc.sync.dma_start(b1t[:], b1[:])
                onest = cpool.tile([1, H], F16, tag="ones")
                nc.gpsimd.memset(onest[:], 1.0)
            MAXNT = max(T) if T else 1

            # ---- Phase A: xp table = gelu(x @ W1 [+ b1]), all NA rows ----
            xpd = xpdp.tile([NA, H], F16, tag="xpd")
            GRP = 4  # node tiles per PSUM bank
            for g0 in range(0, NB, GRP):
                gl = min(GRP, NB - g0)
                xa_t = xap.tile([128, GRP * 128], F16, tag="xa")
                aeng = nc.sync if (g0 // GRP) % 2 == 0 else nc.scalar
                aeng.dma_start(xa_t[:, :gl * 128],
                               xfa[:, g0 * 128:(g0 + gl) * 128])
                ps = pxp.tile([128, GRP * 128], F32, tag="pxp")
                for b in range(gl):
                    nc.tensor.matmul(
                        ps[:, b * 128:(b + 1) * 128],
                        xa_t[:, b * 128:(b + 1) * 128],
                        w1t[:],
                        start=True, stop=not have_b1)
                    if have_b1:
                        nc.tensor.matmul(
                            ps[:, b * 128:(b + 1) * 128],
                            onest[:1, :], b1t[:1, :],
                            start=False, stop=True)
                xp_t = xpsp.tile([128, GRP * 128], F16, tag="xps")
                nc.scalar.activation(xp_t[:, :gl * 128], ps[:, :gl * 128],
                                     act)
                dst3 = xpd[g0 * 128:(g0 + gl) * 128, :] \
                    .rearrange("(b n) h -> n b h", b=gl)
                src3 = xp_t[:, :gl * 128].rearrange("n (b h) -> n b h", b=gl)
                nc.sync.dma_start(dst3, src3)

            # ---- Phase B: R scatter passes into the SBUF aggregator ----
            # (phase-B-only constants load while phase A computes)
            xf_t = cpool.tile([H, NPAD], F16, tag="xfm")
            nc.sync.dma_start(xf_t[:], xfm[:])
            jd_t = cpool.tile([128, GT], F16, tag="jd")
            nc.scalar.dma_start(jd_t[:], jdd[:])
            ix_t = cpool.tile([128, GT * 8], I16, tag="ixd")
            nc.scalar.dma_start(ix_t[:], ixdd[:])
            aggr_t = cpool.tile([128, NPAD], F32, tag="aggr")

            def scatter_pass(r, w, dbg_out=None):
                """Gather+scatter group (r, w); returns PSUM tile or None."""
                g = r * NW + w
                nt = T[g]
                if nt == 0:
                    return None
                go = OFFG[g]
                bas_t = basp.tile([128, nt * H], F16, tag="bas")
                beng = nc.sync if w % 2 == 0 else nc.scalar
                beng.dma_start(bas_t[:], basf[:, go * H:(go + nt) * H])

                v = swq[0]
                swq[0] += 1
                g_t = gatp.tile([128, MAXNT * H], F16, tag="gat")
                g3 = g_t[:, :nt * H].rearrange("p (t e) -> p t e", t=nt)
                nc.gpsimd.dma_gather(
                    g3[:, :, :],
                    xpd[row_start[r]:row_start[r + 1], :],
                    ix_t[:, go * 8:(go + nt) * 8],
                    nt * 128, nt * 128, H,
                    transpose=False, single_packet=False,
                    queue_num=v % NSWQ)

                msg_t = msgp.tile([128, nt * H], F16, tag="msg")
                nc.vector.tensor_mul(msg_t[:], g_t[:, :nt * H], bas_t[:])

                s_t = stp.tile([128, nt * 128], F16, tag="s")
                s3 = s_t[:].rearrange("p (t c) -> p t c", t=nt)
                io_b = iota_t[:].rearrange("p (o c) -> p o c", o=1) \
                    .broadcast_to([128, nt, 128])
                jd_b = jd_t[:, go:go + nt] \
                    .rearrange("p (t o) -> p t o", o=1) \
                    .broadcast_to([128, nt, 128])
                nc.vector.tensor_tensor(s3, io_b, jd_b, IS_EQ)

                ps_ag = pag.tile([128, 128], F32, tag="pag")
                for t in range(nt):
                    nc.tensor.matmul(
                        ps_ag[:],
                        msg_t[:, t * 128:(t + 1) * 128],
                        s_t[:, t * 128:(t + 1) * 128],
                        start=(t == 0), stop=(t == nt - 1))
                if dbg_out:
                    nc.sync.dma_start(dbg_out["g"][:, :], g_t[:])
                    nc.sync.dma_start(dbg_out["msg"][:, :nt * H], msg_t[:])
                    nc.sync.dma_start(dbg_out["s"][:, :nt * 128], s_t[:])
                    agc = ffnp.tile([128, 128], F32, tag="dbgag")
                    nc.vector.tensor_copy(agc[:], ps_ag[:])
                    nc.sync.dma_start(dbg_out["ag"][:, :], agc[:])
                return ps_ag

            # Pass 0: aggr <- psum + x_residual
            for w in range(NW):
                r0 = w * 128
                ps = scatter_pass(0, w, dbg if w == 0 else None)
                if ps is not None:
                    nc.vector.tensor_add(aggr_t[:, r0:r0 + 128], ps[:],
                                         xf_t[:, r0:r0 + 128])
                else:
                    nc.vector.tensor_copy(aggr_t[:, r0:r0 + 128],
                                          xf_t[:, r0:r0 + 128])

            # Passes 1..R-2: aggr += psum
            for r in range(1, R - 1):
                for w in range(NW):
                    r0 = w * 128
                    ps = scatter_pass(r, w)
                    if ps is not None:
                        nc.vector.tensor_add(aggr_t[:, r0:r0 + 128], ps[:],
                                             aggr_t[:, r0:r0 + 128])

            # Pass R-1: x16 = psum + aggr, then FFN + store.
            OGW = 8  # windows per output-store group
            out_g = None
            for w in range(NW):
                r0 = w * 128
                if w % OGW == 0:
                    gw = min(OGW, NW - w)
                    out_g = ogp.tile([H, OGW * 128], F16, tag="og")

                ps = scatter_pass(R - 1, w)
                x16_t = ffnp.tile([128, 128], F16, tag="x16")
                if ps is not None:
                    nc.vector.tensor_add(x16_t[:], ps[:],
                                         aggr_t[:, r0:r0 + 128])
                else:
                    nc.vector.tensor_copy(x16_t[:], aggr_t[:, r0:r0 + 128])

                ps2 = pffn.tile([128, 128], F32, tag="pffn")
                nc.tensor.matmul(ps2[:], w2t[:], x16_t[:],
                                 start=True, stop=True)
                y1_t = ffnp.tile([128, 128], F16, tag="y1")
                nc.scalar.activation(y1_t[:], ps2[:], act, bias=c2t[:, 0:1])
                ps3 = pffn.tile([128, 128], F32, tag="pffn")
                nc.tensor.matmul(ps3[:], w3t[:], y1_t[:],
                                 start=True, stop=True)
                y2_t = ffnp.tile([128, 128], F32, tag="y2")
                nc.scalar.activation(y2_t[:], ps3[:], act, bias=c3t[:, 0:1])
                oc = (w % OGW) * 128
                nc.vector.tensor_add(out_g[:, oc:oc + 128], y2_t[:],
                                     x16_t[:])
                if w % OGW == OGW - 1 or w == NW - 1:
                    g0 = (w // OGW) * OGW * 128
                    nc.sync.dma_start(outd[:, g0:g0 + gw * 128],
                                      out_g[:, :gw * 128])

    nc.compile()
    return nc


def run_compiled(nc, in_maps, meta, ncores=8, **kw):
    from concourse.bass_utils import run_bass_kernel_spmd
    res = run_bass_kernel_spmd(nc, in_maps, list(range(ncores)), **kw)
    N, NSH = meta["N"], meta["NSH"]
    out = np.empty((N, H), np.float32)
    for k in range(ncores):
        out[k * NSH:(k + 1) * NSH] = \
            res.results[k]["out"][:, :NSH].T.astype(np.float32)
    return out, res


def kernel(**inputs):
    inputs = {k: np.asarray(v) for k, v in inputs.items()}
    in_maps, meta = prep_inputs(**inputs)
    nc = build_program(meta)
    out, _ = run_compiled(nc, in_maps, meta)
    return out


# revision 39
# speedup vs baseline: 1.0144x; 1.0144x over previous
"""Trainium2 Bass kernel for gnn_message_passing (nn_Conv_82506321756833).

Computes, for N=50000 nodes / E=800000 edges / H=128:
    xp   = gelu(x @ W1 + b1)
    aggr = segment_sum(xp[src] * bases, dst)
    x    = x_feat + aggr
    y    = gelu(bn1(x @ W2 + b2)); y = gelu(bn2(y @ W3 + b3))
    out  = x + y

Sharding: nodes are partitioned contiguously across 8 cores (graph
parallel); each core owns its node shard and all edges whose dst lands
in the shard.  Edges are bucketed host-side by (src range, dst window):
R=4 source ranges x 128-node destination windows.

Phase A (replicated per core): stream x feature-major, compute
xp = gelu(x@W1+b1) for ALL nodes, store as a row table [NA, H] f16 in
device DRAM (a DRAM tile, so the tile framework tracks the per-range
RAW dependencies into the gathers via AP intersection).

Phase B runs R scatter passes; pass r only needs xp rows of range r, so
pass-0 gathers start as soon as the (smallest) first range of phase A
is stored.
Per (range, window) group:
  1. dma_gather xp rows EDGE-major (transpose=False; transpose gathers
     corrupt on swdge queues != 0) on 4 SWDGE queues round-robin.
     Pad slots gather row 0 of the range (bases there are zero); a
     host-side greedy balancer packs every (window, range) cell to just
     under its tile multiple so padding stays ~3%.
  2. multiply by the (host-presorted, edge-major) bases tile -> msg;
  3. scatter-sum via one-hot matmuls msg.T @ S into PSUM; S is built by
     one DVE is_equal per group comparing a broadcast iota row against
     the per-edge dst-slot id;
  4. accumulate PSUM into an SBUF aggregator strip [128, NPAD] f32
     (pass 0 also folds in the residual x).
The final pass fuses the 2-layer FFN (BN folded into W2/W3 + bias) and
streams outputs to HBM in fp16, one store per 8-window group.
"""

import numpy as np

import concourse.bacc as bacc
import concourse.tile as tile
from concourse import mybir

H = 128
WIN = 128
R = 4  # source ranges (rows per range < 32768 for int16 gather indices)
DEBUG_DUMP = False
PRIME_VAL = 0.0
NSWQ = 4  # SWDGE queues for gathers
SCRATCH = 49152  # SWDGE descriptor scratch (bytes/partition)
BN_EPS = 1e-5
F16 = mybir.dt.float16
F32 = mybir.dt.float32
I16 = mybir.dt.int16
U32 = mybir.dt.uint32
GELU = mybir.ActivationFunctionType.Gelu
IS_EQ = mybir.AluOpType.is_equal


def _ceil_to(x, m):
    return (x + m - 1) // m * m


def _wrap16(idx):
    """[L] int16 index list -> [128, L//16] wrapped+replicated."""
    L = idx.shape[0]
    m = idx.reshape(L // 16, 16).T  # [16, L/16]
    return np.ascontiguousarray(np.tile(m, (8, 1)))  # [128, L/16]


TPW = (3, 4, 5, 5)  # target tiles per (window, range) group: ~6% slack
#  per range for the greedy; range 0 smallest so phase A unblocks the
#  first gathers as early as possible


def _range_parts(NB):
    wts = np.array(TPW, np.float64)
    wts /= wts.sum()
    parts = np.floor(NB * wts).astype(np.int64)
    parts[0] += NB - parts.sum()
    starts = np.zeros(R + 1, np.int64)
    np.cumsum(parts, out=starts[1:])
    return parts.tolist(), starts * 128  # tile counts, row starts


def prep_inputs(x_feat, bases, src, dst, W1, b1, W2, b2, W3, b3,
                g1, be1, m1, v1, g2, be2, m2, v2, ncores=8):
    """Host-side sharding: bucket edges by (src range, dst window), build
    per-core tile-grid inputs (bases, slot ids, gather indices, counts)."""
    N = x_feat.shape[0]
    assert N % ncores == 0
    NSH = N // ncores
    NW = (NSH + WIN - 1) // WIN
    NPAD = NW * WIN
    NA = _ceil_to(N, 128)
    NB = NA // 128
    parts, row_start = _range_parts(NB)
    assert max(parts) * 128 <= 32768

    x_feat = np.asarray(x_feat, np.float32)
    bases = np.asarray(bases, np.float32)
    src = np.asarray(src, np.int64)
    dst = np.asarray(dst, np.int64)

    NG = R * NW

    # Greedy range assignment (quota-normalized): place each node's rows in
    # the range where its (core, window) cells stay lowest relative to fair
    # share, flattening the max-over-cores per-cell counts the shared
    # program pads to tile multiples.
    cell = (dst // NSH) * NW + (dst % NSH) // WIN
    order0 = np.argsort(src, kind="stable")
    s_sorted = src[order0]
    c_sorted = cell[order0]
    starts = np.searchsorted(s_sorted, np.arange(N))
    ends = np.searchsorted(s_sorted, np.arange(N) + 1)
    tot = np.bincount(cell, minlength=ncores * NW).astype(np.float64)
    qr = np.array([p / float(NB) for p in parts])  # range share
    quota = np.maximum(tot[None, :] * qr[:, None], 1.0)  # [R, cells]
    cnt_rc = np.zeros((R, ncores * NW))
    used = np.zeros(R, np.int64)
    cap = np.array([p * 128 for p in parts], np.int64)
    assign = np.zeros(N, np.int8)
    for n in np.argsort(-(ends - starts), kind="stable"):
        uc, mult = np.unique(c_sorted[starts[n]:ends[n]], return_counts=True)
        if len(uc):
            d = ((cnt_rc[:, uc] + mult[None, :]) / quota[:, uc]).max(axis=1)
        else:
            d = np.zeros(R)
        d = np.where(used < cap, d, np.inf)
        rpick = int(np.argmin(d))
        assign[n] = rpick
        used[rpick] += 1
        if len(uc):
            cnt_rc[rpick, uc] += mult
    rowof = np.empty(N, np.int64)
    for rr in range(R):
        ids = np.nonzero(assign == rr)[0]
        rowof[ids] = row_start[rr] + np.arange(len(ids))

    xfa = np.zeros((H, NA), np.float16)
    xfa[:, rowof] = x_feat.T.astype(np.float16)

    w1h = np.ascontiguousarray(np.asarray(W1, np.float32).astype(np.float16))
    a1 = (np.asarray(g1, np.float32) /
          np.sqrt(np.asarray(v1, np.float32) + BN_EPS))
    a2 = (np.asarray(g2, np.float32) /
          np.sqrt(np.asarray(v2, np.float32) + BN_EPS))
    w2f = np.ascontiguousarray((np.asarray(W2, np.float32) * a1[None, :])
                               .astype(np.float16))
    w3f = np.ascontiguousarray((np.asarray(W3, np.float32) * a2[None, :])
                               .astype(np.float16))
    c2 = ((np.asarray(b2, np.float32) - np.asarray(m1, np.float32)) * a1
          + np.asarray(be1, np.float32)).astype(np.float32).reshape(H, 1)
    c3 = ((np.asarray(b3, np.float32) - np.asarray(m2, np.float32)) * a2
          + np.asarray(be2, np.float32)).astype(np.float32).reshape(H, 1)
    have_b1 = bool(np.any(np.asarray(b1)))
    b1h = np.asarray(b1, np.float32).astype(np.float16).reshape(1, H)

    # Pass 1: per-core bucketing by (range, window) in range-major order.
    core_of = dst // NSH
    percore = []
    cnt_all = np.zeros((ncores, NG), np.int64)
    for k in range(ncores):
        sel = np.nonzero(core_of == k)[0]
        ld = dst[sel] - k * NSH
        w = ld // WIN
        j = ld % WIN
        s = rowof[src[sel]]
        r = assign[src[sel]].astype(np.int64)
        key = r * NW + w
        order = np.lexsort((s, key))
        w, j, s, r, key, sel = (w[order], j[order], s[order], r[order],
                                key[order], sel[order])
        cntg = np.bincount(key, minlength=NG)
        cnt_all[k] = cntg
        starts2 = np.zeros(NG, np.int64)
        np.cumsum(cntg[:-1], out=starts2[1:])
        rank = np.arange(len(key)) - starts2[key]
        percore.append((key, j, s, r, rank, sel))

    # Shared (max-over-cores) tile grid: all cores run one program.
    T = (cnt_all.max(axis=0) + 127) // 128  # [NG]
    OFFG = np.zeros(NG + 1, np.int64)
    np.cumsum(T, out=OFFG[1:])
    GT = int(OFFG[-1])

    # Pass 2: per-core arrays in the shared grid.
    in_maps = []
    for k in range(ncores):
        key, j, s, r, rank, sel = percore[k]
        gt = OFFG[key] + rank // 128
        p = rank % 128

        basf = np.zeros((128, GT * H), np.float16)
        basf.reshape(128, GT, H)[p, gt, :] = bases[sel].astype(np.float16)
        jd = np.full((128, GT), -1, np.float16)
        jd[p, gt] = j.astype(np.float16)

        ixflat = np.zeros(GT * 128, np.int16)
        ixflat[gt * 128 + p] = (s - row_start[r]).astype(np.int16)
        ixd = np.zeros((128, GT * 8), np.int16)
        for g in range(NG):
            if T[g] == 0:
                continue
            seg = ixflat[OFFG[g] * 128:(OFFG[g] + T[g]) * 128]
            ixd[:, OFFG[g] * 8:(OFFG[g] + T[g]) * 8] = _wrap16(seg)

        xfm = np.zeros((H, NPAD), np.float16)
        xfm[:, :NSH] = x_feat[k * NSH:(k + 1) * NSH].T.astype(np.float16)

        maps = dict(xfa=xfa, basf=basf, jd=jd, ixd=ixd,
                    w1=w1h, w2=w2f, w3=w3f, c2=c2, c3=c3, xfm=xfm)
        if have_b1:
            maps["b1"] = b1h
        in_maps.append(maps)

    meta = dict(N=N, NSH=NSH, NW=NW, NPAD=NPAD, NA=NA, NB=NB,
                T=T.tolist(), OFFG=OFFG.tolist(), GT=GT,
                parts=parts, row_start=row_start.tolist(),
                have_b1=have_b1)
    return in_maps, meta


def build_program(meta, ncores=8, act=GELU):
    NA, NW, NPAD, NB = meta["NA"], meta["NW"], meta["NPAD"], meta["NB"]
    T, OFFG, GT = meta["T"], meta["OFFG"], meta["GT"]
    row_start = meta["row_start"]
    have_b1 = meta["have_b1"]

    nc = bacc.Bacc("TRN2", target_bir_lowering=False, debug=False,
                   num_devices=ncores, num_swdge_queues=NSWQ,
                   dynamic_dma_scratch_size=SCRATCH)
    xfa = nc.dram_tensor("xfa", [H, NA], F16, kind="ExternalInput").ap()
    xfm = nc.dram_tensor("xfm", [H, NPAD], F16, kind="ExternalInput").ap()
    basf = nc.dram_tensor("basf", [128, GT * H], F16,
                          kind="ExternalInput").ap()
    jdd = nc.dram_tensor("jd", [128, GT], F16, kind="ExternalInput").ap()
    ixdd = nc.dram_tensor("ixd", [128, GT * 8], I16,
                          kind="ExternalInput").ap()
    w1 = nc.dram_tensor("w1", [H, H], F16, kind="ExternalInput").ap()
    w2 = nc.dram_tensor("w2", [H, H], F16, kind="ExternalInput").ap()
    w3 = nc.dram_tensor("w3", [H, H], F16, kind="ExternalInput").ap()
    c2 = nc.dram_tensor("c2", [H, 1], F32, kind="ExternalInput").ap()
    c3 = nc.dram_tensor("c3", [H, 1], F32, kind="ExternalInput").ap()
    b1 = (nc.dram_tensor("b1", [1, H], F16, kind="ExternalInput").ap()
          if have_b1 else None)
    outd = nc.dram_tensor("out", [H, NPAD], F16, kind="ExternalOutput").ap()

    dbg = {}
    if DEBUG_DUMP:
        MAXNT0 = max(T) if T else 1
        dbg["g"] = nc.dram_tensor("dbg_g", [128, MAXNT0 * H], F16,
                                  kind="ExternalOutput").ap()
        dbg["msg"] = nc.dram_tensor("dbg_msg", [128, MAXNT0 * H], F16,
                                    kind="ExternalOutput").ap()
        dbg["s"] = nc.dram_tensor("dbg_s", [128, MAXNT0 * 128], F16,
                                  kind="ExternalOutput").ap()
        dbg["ag"] = nc.dram_tensor("dbg_ag", [128, 128], F32,
                                   kind="ExternalOutput").ap()

    swq = [0]

    with tile.TileContext(nc) as tc:
        with (
            tc.tile_pool(name="const", bufs=1) as cpool,
            tc.tile_pool(name="xpd", bufs=1, space="DRAM") as xpdp,
            tc.tile_pool(name="xa", bufs=4) as xap,
            tc.tile_pool(name="xps", bufs=4) as xpsp,
            tc.tile_pool(name="bas", bufs=6) as basp,
            tc.tile_pool(name="gat", bufs=12) as gatp,
            tc.tile_pool(name="msg", bufs=8) as msgp,
            tc.tile_pool(name="st", bufs=8) as stp,
            tc.tile_pool(name="ffn", bufs=2) as ffnp,
            tc.tile_pool(name="og", bufs=2) as ogp,
            tc.tile_pool(name="pxp", bufs=3, space="PSUM") as pxp,
            tc.tile_pool(name="pag", bufs=3, space="PSUM") as pag,
            tc.tile_pool(name="pffn", bufs=2, space="PSUM") as pffn,
        ):
            # constants / resident inputs (big ones on the idle vector queue)
            w1t = cpool.tile([H, H], F16, tag="w1")
            nc.sync.dma_start(w1t[:], w1[:])
            w2t = cpool.tile([H, H], F16, tag="w2")
            nc.sync.dma_start(w2t[:], w2[:])
            w3t = cpool.tile([H, H], F16, tag="w3")
            nc.sync.dma_start(w3t[:], w3[:])
            c2t = cpool.tile([H, 1], F32, tag="c2")
            nc.sync.dma_start(c2t[:], c2[:])
            c3t = cpool.tile([H, 1], F32, tag="c3")
            nc.sync.dma_start(c3t[:], c3[:])
            iota_t = cpool.tile([128, 128], F16, tag="iota")
            nc.gpsimd.iota(iota_t[:], [[1, 128]], channel_multiplier=0,
                           allow_small_or_imprecise_dtypes=True)
            if have_b1:
                b1t = cpool.tile([1, H], F16, tag="b1")
                nc.sync.dma_start(b1t[:], b1[:])
                onest = cpool.tile([1, H], F16, tag="ones")
                nc.gpsimd.memset(onest[:], 1.0)
            MAXNT = max(T) if T else 1

            # ---- Phase A: xp table = gelu(x @ W1 [+ b1]), all NA rows ----
            xpd = xpdp.tile([NA, H], F16, tag="xpd")
            GRP = 4  # node tiles per PSUM bank
            for g0 in range(0, NB, GRP):
                gl = min(GRP, NB - g0)
                xa_t = xap.tile([128, GRP * 128], F16, tag="xa")
                nc.scalar.dma_start(xa_t[:, :gl * 128],
                                    xfa[:, g0 * 128:(g0 + gl) * 128])
                ps = pxp.tile([128, GRP * 128], F32, tag="pxp")
                for b in range(gl):
                    nc.tensor.matmul(
                        ps[:, b * 128:(b + 1) * 128],
                        xa_t[:, b * 128:(b + 1) * 128],
                        w1t[:],
                        start=True, stop=not have_b1)
                    if have_b1:
                        nc.tensor.matmul(
                            ps[:, b * 128:(b + 1) * 128],
                            onest[:1, :], b1t[:1, :],
                            start=False, stop=True)
                xp_t = xpsp.tile([128, GRP * 128], F16, tag="xps")
                nc.scalar.activation(xp_t[:, :gl * 128], ps[:, :gl * 128],
                                     act)
                dst3 = xpd[g0 * 128:(g0 + gl) * 128, :] \
                    .rearrange("(b n) h -> n b h", b=gl)
                src3 = xp_t[:, :gl * 128].rearrange("n (b h) -> n b h", b=gl)
                nc.scalar.dma_start(dst3, src3)

            # ---- Phase B: R scatter passes into the SBUF aggregator ----
            # (phase-B-only constants load while phase A computes)
            xf_t = cpool.tile([H, NPAD], F16, tag="xfm")
            nc.sync.dma_start(xf_t[:], xfm[:])
            jd_t = cpool.tile([128, GT], F16, tag="jd")
            nc.sync.dma_start(jd_t[:], jdd[:])
            ix_t = cpool.tile([128, GT * 8], I16, tag="ixd")
            nc.sync.dma_start(ix_t[:], ixdd[:])
            aggr_t = cpool.tile([128, NPAD], F32, tag="aggr")

            def scatter_pass(r, w, dbg_out=None):
                """Gather+scatter group (r, w); returns PSUM tile or None."""
                g = r * NW + w
                nt = T[g]
                if nt == 0:
                    return None
                go = OFFG[g]
                bas_t = basp.tile([128, nt * H], F16, tag="bas")
                nc.sync.dma_start(bas_t[:], basf[:, go * H:(go + nt) * H])

                v = swq[0]
                swq[0] += 1
                g_t = gatp.tile([128, MAXNT * H], F16, tag="gat")
                g3 = g_t[:, :nt * H].rearrange("p (t e) -> p t e", t=nt)
                nc.gpsimd.dma_gather(
                    g3[:, :, :],
                    xpd[row_start[r]:row_start[r + 1], :],
                    ix_t[:, go * 8:(go + nt) * 8],
                    nt * 128, nt * 128, H,
                    transpose=False, single_packet=False,
                    queue_num=v % NSWQ)

                msg_t = msgp.tile([128, nt * H], F16, tag="msg")
                nc.vector.tensor_mul(msg_t[:], g_t[:, :nt * H], bas_t[:])

                s_t = stp.tile([128, nt * 128], F16, tag="s")
                s3 = s_t[:].rearrange("p (t c) -> p t c", t=nt)
                io_b = iota_t[:].rearrange("p (o c) -> p o c", o=1) \
                    .broadcast_to([128, nt, 128])
                jd_b = jd_t[:, go:go + nt] \
                    .rearrange("p (t o) -> p t o", o=1) \
                    .broadcast_to([128, nt, 128])
                nc.vector.tensor_tensor(s3, io_b, jd_b, IS_EQ)

                ps_ag = pag.tile([128, 128], F32, tag="pag")
                for t in range(nt):
                    nc.tensor.matmul(
                        ps_ag[:],
                        msg_t[:, t * 128:(t + 1) * 128],
                        s_t[:, t * 128:(t + 1) * 128],
                        start=(t == 0), stop=(t == nt - 1))
                if dbg_out:
                    nc.sync.dma_start(dbg_out["g"][:, :], g_t[:])
                    nc.sync.dma_start(dbg_out["msg"][:, :nt * H], msg_t[:])
                    nc.sync.dma_start(dbg_out["s"][:, :nt * 128], s_t[:])
                    agc = ffnp.tile([128, 128], F32, tag="dbgag")
                    nc.vector.tensor_copy(agc[:], ps_ag[:])
                    nc.sync.dma_start(dbg_out["ag"][:, :], agc[:])
                return ps_ag

            # Pass 0: aggr <- psum + x_residual
            for w in range(NW):
                r0 = w * 128
                ps = scatter_pass(0, w, dbg if w == 0 else None)
                if ps is not None:
                    nc.vector.tensor_add(aggr_t[:, r0:r0 + 128], ps[:],
                                         xf_t[:, r0:r0 + 128])
                else:
                    nc.vector.tensor_copy(aggr_t[:, r0:r0 + 128],
                                          xf_t[:, r0:r0 + 128])

            # Passes 1..R-2: aggr += psum
            for r in range(1, R - 1):
                for w in range(NW):
                    r0 = w * 128
                    ps = scatter_pass(r, w)
                    if ps is not None:
                        nc.vector.tensor_add(aggr_t[:, r0:r0 + 128], ps[:],
                                             aggr_t[:, r0:r0 + 128])

            # Pass R-1: x16 = psum + aggr, then FFN + store.
            OGW = 8  # windows per output-store group
            out_g = None
            for w in range(NW):
                r0 = w * 128
                if w % OGW == 0:
                    gw = min(OGW, NW - w)
                    out_g = ogp.tile([H, OGW * 128], F16, tag="og")

                ps = scatter_pass(R - 1, w)
                x16_t = ffnp.tile([128, 128], F16, tag="x16")
                if ps is not None:
                    nc.vector.tensor_add(x16_t[:], ps[:],
                                         aggr_t[:, r0:r0 + 128])
                else:
                    nc.vector.tensor_copy(x16_t[:], aggr_t[:, r0:r0 + 128])

                ps2 = pffn.tile([128, 128], F32, tag="pffn")
                nc.tensor.matmul(ps2[:], w2t[:], x16_t[:],
                                 start=True, stop=True)
                y1_t = ffnp.tile([128, 128], F16, tag="y1")
                nc.scalar.activation(y1_t[:], ps2[:], act, bias=c2t[:, 0:1])
                ps3 = pffn.tile([128, 128], F32, tag="pffn")
                nc.tensor.matmul(ps3[:], w3t[:], y1_t[:],
                                 start=True, stop=True)
                y2_t = ffnp.tile([128, 128], F32, tag="y2")
                nc.scalar.activation(y2_t[:], ps3[:], act, bias=c3t[:, 0:1])
                oc = (w % OGW) * 128
                nc.vector.tensor_add(out_g[:, oc:oc + 128], y2_t[:],
                                     x16_t[:])
                if w % OGW == OGW - 1 or w == NW - 1:
                    g0 = (w // OGW) * OGW * 128
                    nc.sync.dma_start(outd[:, g0:g0 + gw * 128],
                                      out_g[:, :gw * 128])

    nc.compile()
    return nc


def run_compiled(nc, in_maps, meta, ncores=8, **kw):
    from concourse.bass_utils import run_bass_kernel_spmd
    res = run_bass_kernel_spmd(nc, in_maps, list(range(ncores)), **kw)
    N, NSH = meta["N"], meta["NSH"]
    out = np.empty((N, H), np.float32)
    for k in range(ncores):
        out[k * NSH:(k + 1) * NSH] = \
            res.results[k]["out"][:, :NSH].T.astype(np.float32)
    return out, res


def kernel(**inputs):
    inputs = {k: np.asarray(v) for k, v in inputs.items()}
    in_maps, meta = prep_inputs(**inputs)
    nc = build_program(meta)
    out, _ = run_compiled(nc, in_maps, meta)
    return out
